# revision 12
# baseline (speedup 1.0000x reference)
"""Bass/Trainium2 kernel for nn_Attention (B=4, N=2048, IN=256, HID=1024,
D=1024, OUT=256, H=8 heads), SPMD over 8 NeuronCores.

Sharding: core c handles batch b = c//2 and head-group g = c%2 (4 heads,
512 of the 1024 inner features).  Layer-1 of each QKV MLP is recomputed on
both cores of a batch (cheap); the output projection is computed per
head-group and the two partial products are summed on the host (plus bias;
the query mask commutes with the projection so it is applied on host too).

Per-core dataflow (laid out so no on-chip transposes are ever needed):
  xT (256,2048) -> L1 feature-major h=(1024,2048) tanh -> L2:
     qT,kT feature-major (512,2048) = 4 head tiles [128,2048]
     v token-major (2048,512) (bias added via rank-1 matmul)
  attention per (head, q-chunk of 1024): S^T tiles [128 k-tok, 1024 q]
     = kT_tile.T @ qT ; key mask enters as the per-partition bias of the Exp
     activation; the no-self-attention diagonal is one [128,128] additive
     DVE op; denominators s = ones^T @ (sum of exp tiles); 1/s is broadcast
     across partitions with an SBUF->SBUF DMA; y^T accumulates in PSUM.
  proj: out^T = Wp_g^T @ (y^T * 1/s) in bf16.
"""

import numpy as np

B, N, IN_DIM, HID, D, OUT_DIM, H = 4, 2048, 256, 1024, 1024, 256, 8
NCORES = 8
HG = 2                 # head groups (cores per batch)
DG = D // HG           # 512 features per group
HEADS_G = H // HG      # 4 heads per core
Dh = D // H            # 128
NEG = -30000.0         # additive mask value (exp underflows to 0)

_CACHE = {}


def _build_nc():
    import concourse.mybir as mybir
    import concourse.tile as tile
    from concourse import bacc
    from contextlib import ExitStack

    dt = mybir.dt
    f32 = dt.float32
    f32r = dt.float32r
    bf16 = dt.bfloat16
    AF = mybir.ActivationFunctionType
    ALU = mybir.AluOpType

    nc = bacc.Bacc("TRN2", target_bir_lowering=False, debug=False)

    # ---- DRAM I/O ----
    xqT = nc.dram_tensor("xqT", [IN_DIM, N], f32r, kind="ExternalInput")
    xkT = nc.dram_tensor("xkT", [IN_DIM, N], f32r, kind="ExternalInput")
    xvT = nc.dram_tensor("xvT", [IN_DIM, N], f32r, kind="ExternalInput")
    wq1 = nc.dram_tensor("wq1", [IN_DIM, HID], f32r, kind="ExternalInput")
    wk1 = nc.dram_tensor("wk1", [IN_DIM, HID], f32r, kind="ExternalInput")
    wv1 = nc.dram_tensor("wv1", [IN_DIM, HID], f32r, kind="ExternalInput")
    bq1 = nc.dram_tensor("bq1", [128, HID // 128], f32, kind="ExternalInput")
    bk1 = nc.dram_tensor("bk1", [128, HID // 128], f32, kind="ExternalInput")
    bv1 = nc.dram_tensor("bv1", [128, HID // 128], f32, kind="ExternalInput")
    wq2 = nc.dram_tensor("wq2", [HID, DG], f32r, kind="ExternalInput")
    wk2 = nc.dram_tensor("wk2", [HID, DG], f32r, kind="ExternalInput")
    wv2 = nc.dram_tensor("wv2", [HID, DG], f32r, kind="ExternalInput")
    bq2 = nc.dram_tensor("bq2", [128, DG // 128], f32, kind="ExternalInput")
    bk2 = nc.dram_tensor("bk2", [128, DG // 128], f32, kind="ExternalInput")
    bv2r = nc.dram_tensor("bv2r", [128, DG], f32r, kind="ExternalInput")
    onesc = nc.dram_tensor("onesc", [128, 1], f32r, kind="ExternalInput")
    e0Td = nc.dram_tensor("e0Td", [128, 128], f32r, kind="ExternalInput")
    wpb = nc.dram_tensor("wpb", [DG, OUT_DIM], bf16, kind="ExternalInput")
    kmadd = nc.dram_tensor("kmadd", [128, N // 128], f32, kind="ExternalInput")
    dneg = nc.dram_tensor("dneg", [128, 128], f32, kind="ExternalInput")
    outT = nc.dram_tensor("outT", [OUT_DIM, N], f32, kind="ExternalOutput")

    KT1 = IN_DIM // 128          # 2  k-tiles in layer 1
    KT2 = HID // 128             # 8  k-tiles in layer 2
    MT1 = HID // 128             # 8  m-tiles in layer 1
    NTOK = N // 128              # 16 token tiles
    NQCH = 2                     # q-chunks
    QW = N // NQCH               # 1024

    with tile.TileContext(nc) as tc, ExitStack() as ctx:
        # pools (PSUM: ps 3x2 banks + psy 1x2 banks = 8 banks)
        ps = ctx.enter_context(tc.tile_pool(name="ps", bufs=3, space="PSUM"))
        psy = ctx.enter_context(tc.tile_pool(name="psy", bufs=1, space="PSUM"))
        singles = ctx.enter_context(tc.tile_pool(name="singles", bufs=1))
        xt_pool = ctx.enter_context(tc.tile_pool(name="xt", bufs=3))
        w1_pool = ctx.enter_context(tc.tile_pool(name="w1", bufs=4))
        w2_pool = ctx.enter_context(tc.tile_pool(name="w2", bufs=8))
        h_pool = ctx.enter_context(tc.tile_pool(name="h", bufs=8))
        qk_pool = ctx.enter_context(tc.tile_pool(name="qk", bufs=8))
        v_pool = ctx.enter_context(tc.tile_pool(name="v", bufs=4))
        pt_pool = ctx.enter_context(tc.tile_pool(name="pt", bufs=4))
        sacc_pool = ctx.enter_context(tc.tile_pool(name="sacc", bufs=2))
        ysc_pool = ctx.enter_context(tc.tile_pool(name="ysc", bufs=4))
        rm_pool = ctx.enter_context(tc.tile_pool(name="rm", bufs=2))
        rb_pool = ctx.enter_context(tc.tile_pool(name="rb", bufs=2))
        rmd_pool = ctx.enter_context(
            tc.tile_pool(name="rmd", bufs=2, space="DRAM"))
        out_pool = ctx.enter_context(tc.tile_pool(name="out", bufs=1))

        # constants
        ones_col = singles.tile([128, 1], f32r, tag="ones_col")
        nc.sync.dma_start(out=ones_col, in_=onesc[:, :])
        e0T = singles.tile([128, 128], f32r, tag="e0T")
        nc.sync.dma_start(out=e0T, in_=e0Td[:, :])
        km_sb = singles.tile([128, N // 128], f32, tag="km")
        nc.sync.dma_start(out=km_sb, in_=kmadd[:, :])
        dneg_sb = singles.tile([128, 128], f32, tag="dneg")
        nc.sync.dma_start(out=dneg_sb, in_=dneg[:, :])
        bv2_sb = singles.tile([128, DG], f32r, tag="bv2")
        nc.sync.dma_start(out=bv2_sb, in_=bv2r[:, :])
        wp_sb = singles.tile([128, HEADS_G, OUT_DIM], bf16, tag="wp")
        nc.sync.dma_start(
            out=wp_sb, in_=wpb.rearrange("(h p) o -> p h o", p=128)
        )
        b1_sb = {}
        b2_sb = {}
        for t, (b1d, b2d) in {
            "q": (bq1, bq2), "k": (bk1, bk2), "v": (bv1, None)
        }.items():
            b1_sb[t] = singles.tile([128, HID // 128], f32, tag=f"b1{t}", name=f"b1{t}")
            nc.sync.dma_start(out=b1_sb[t], in_=b1d[:, :])
            if b2d is not None:
                b2_sb[t] = singles.tile([128, DG // 128], f32, tag=f"b2{t}", name=f"b2{t}")
                nc.sync.dma_start(out=b2_sb[t], in_=b2d[:, :])

        # persistent activations
        qT = [qk_pool.tile([128, N], f32r, tag="qk", name=f"qT{i}") for i in range(HEADS_G)]
        kT = [qk_pool.tile([128, N], f32r, tag="qk", name=f"kT{i}") for i in range(HEADS_G)]
        v_sb = [v_pool.tile([128, 4 * DG], bf16, tag="v", name=f"v{i}") for i in range(4)]

        # ---------------- phase A: the three MLPs ----------------
        for t, xd, w1d, w2d in (
            ("q", xqT, wq1, wq2), ("k", xkT, wk1, wk2), ("v", xvT, wv1, wv2)
        ):
            w1_sb = []
            for k in range(KT1):
                w1t = w1_pool.tile([128, HID], f32r, tag="w1")
                nc.sync.dma_start(out=w1t, in_=w1d[k * 128:(k + 1) * 128, :])
                w1_sb.append(w1t)
            w2_sb = []
            for k in range(KT2):
                w2t = w2_pool.tile([128, DG], f32r, tag="w2")
                nc.sync.dma_start(out=w2t, in_=w2d[k * 128:(k + 1) * 128, :])
                w2_sb.append(w2t)

            for th in range(2):                      # token halves of 1024
                tok_sl = slice(th * QW, (th + 1) * QW)
                xts = []
                for k in range(KT1):
                    xt = xt_pool.tile([128, QW], f32r, tag="xt")
                    nc.sync.dma_start(
                        out=xt, in_=xd[k * 128:(k + 1) * 128, tok_sl]
                    )
                    xts.append(xt)
                # layer 1 (feature-major)
                h_sb = []
                for m in range(MT1):
                    p1 = ps.tile([128, QW], f32, tag="ps")
                    for k in range(KT1):
                        for qc in range(QW // 512):
                            nc.tensor.matmul(
                                p1[:, qc * 512:(qc + 1) * 512],
                                w1_sb[k][:, m * 128:(m + 1) * 128],
                                xts[k][:, qc * 512:(qc + 1) * 512],
                                start=(k == 0), stop=(k == KT1 - 1),
                            )
                    ht = h_pool.tile([128, QW], f32r, tag="h")
                    nc.scalar.activation(
                        out=ht, in_=p1, func=AF.Tanh,
                        bias=b1_sb[t][:, m:m + 1], scale=1.0,
                    )
                    h_sb.append(ht)
                # layer 2
                if t in ("q", "k"):
                    dst = qT if t == "q" else kT
                    for m in range(DG // 128):       # head tiles
                        p2 = ps.tile([128, QW], f32, tag="ps")
                        for k in range(KT2):
                            for qc in range(QW // 512):
                                nc.tensor.matmul(
                                    p2[:, qc * 512:(qc + 1) * 512],
                                    w2_sb[k][:, m * 128:(m + 1) * 128],
                                    h_sb[k][:, qc * 512:(qc + 1) * 512],
                                    start=(k == 0), stop=(k == KT2 - 1),
                                )
                        nc.vector.tensor_scalar_add(
                            out=dst[m][:, tok_sl], in0=p2,
                            scalar1=b2_sb[t][:, m:m + 1],
                        )
                else:
                    # v: token-major [tok, feat], bias via rank-1 matmul
                    for tp in range(4):              # pairs of token tiles
                        pv = ps.tile([128, QW], f32, tag="ps")
                        for tt in range(2):
                            sl = slice(tt * 512, (tt + 1) * 512)
                            for k in range(KT2):
                                nc.tensor.matmul(
                                    pv[:, sl],
                                    h_sb[k][:, (tp * 2 + tt) * 128:
                                            (tp * 2 + tt + 1) * 128],
                                    w2_sb[k][:, :],
                                    start=(k == 0), stop=False,
                                )
                            nc.tensor.matmul(
                                pv[:, sl], e0T[:, :], bv2_sb[:, :],
                                start=False, stop=True,
                            )
                        tok0 = th * 8 + tp * 2
                        nc.vector.tensor_copy(
                            out=v_sb[tok0 // 4][
                                :, (tok0 % 4) * 512:(tok0 % 4 + 2) * 512],
                            in_=pv,
                        )

        # ---------------- phase B: attention + projection ----------------
        for qch in range(NQCH):
            q_sl = [slice(qch * QW + i * 512, qch * QW + (i + 1) * 512)
                    for i in range(QW // 512)]
            ysc_tiles = []
            for hd in range(HEADS_G):
                y2 = psy.tile([128, QW], f32, tag="y2")
                sacc = sacc_pool.tile([128, QW], f32r, tag="sacc")
                for kt in range(NTOK):
                    st = ps.tile([128, QW], f32, tag="ps")
                    for i in range(QW // 512):
                        nc.tensor.matmul(
                            st[:, i * 512:(i + 1) * 512],
                            kT[hd][:, kt * 128:(kt + 1) * 128],
                            qT[hd][:, q_sl[i]],
                            start=True, stop=True,
                        )
                    off = kt * 128 - qch * QW
                    if 0 <= off < QW:
                        nc.vector.tensor_tensor(
                            st[:, off:off + 128], st[:, off:off + 128],
                            dneg_sb, ALU.add,
                        )
                    pt = pt_pool.tile([128, QW], bf16, tag="pt")
                    nc.scalar.activation(
                        out=pt, in_=st, func=AF.Exp,
                        bias=km_sb[:, kt:kt + 1], scale=1.0,
                    )
                    if kt == 0:
                        nc.vector.tensor_copy(out=sacc, in_=pt)
                    else:
                        nc.vector.tensor_tensor(sacc, sacc, pt, ALU.add)
                    vt = v_sb[kt // 4][
                        :, (kt % 4) * 512 + hd * 128:
                        (kt % 4) * 512 + (hd + 1) * 128]
                    for i in range(QW // 512):
                        nc.tensor.matmul(
                            y2[:, i * 512:(i + 1) * 512], vt,
                            pt[:, i * 512:(i + 1) * 512],
                            start=(kt == 0), stop=(kt == NTOK - 1),
                        )
                # denominators: s = ones^T @ sacc  -> [1, QW]
                aux = ps.tile([128, QW], f32, tag="ps")
                for i in range(QW // 512):
                    nc.tensor.matmul(
                        aux[0:1, i * 512:(i + 1) * 512], ones_col[:, :],
                        sacc[:, i * 512:(i + 1) * 512],
                        start=True, stop=True,
                    )
                # broadcast s across partitions (via DRAM bounce), then
                # reciprocal on all 128 lanes (a [1,QW] reciprocal would run
                # on a single DVE lane at ~6.5us)
                rm = rm_pool.tile([1, QW], f32, tag="rm")
                nc.vector.tensor_copy(out=rm, in_=aux[0:1, :])
                rmd = rmd_pool.tile([1, QW], f32, tag="rmd")
                nc.sync.dma_start(out=rmd, in_=rm)
                rb = rb_pool.tile([128, QW], f32, tag="rb")
                nc.sync.dma_start(out=rb, in_=rmd.to_broadcast((128, QW)))
                rb2 = rb_pool.tile([128, QW], f32, tag="rb")
                nc.vector.reciprocal(out=rb2, in_=rb)
                ysc = ysc_pool.tile([128, QW], bf16, tag="ysc")
                nc.vector.tensor_tensor(ysc, y2, rb2, ALU.mult)
                ysc_tiles.append(ysc)
            # projection for this q-chunk
            for od in range(OUT_DIM // 128):
                pp = psy.tile([128, QW], f32, tag="y2")
                for i in range(QW // 512):
                    for hd in range(HEADS_G):
                        nc.tensor.matmul(
                            pp[:, i * 512:(i + 1) * 512],
                            wp_sb[:, hd, od * 128:(od + 1) * 128],
                            ysc_tiles[hd][:, i * 512:(i + 1) * 512],
                            start=(hd == 0), stop=(hd == HEADS_G - 1),
                        )
                ot = out_pool.tile([128, QW], f32, tag="out")
                nc.vector.tensor_copy(out=ot, in_=pp)
                nc.sync.dma_start(
                    out=outT[od * 128:(od + 1) * 128,
                             qch * QW:(qch + 1) * QW],
                    in_=ot,
                )

    nc.compile()
    return nc


def _row0_pad(row, nrows):
    out = np.zeros((nrows, row.shape[0]), np.float32)
    out[0] = row
    return out


def _e0t():
    out = np.zeros((128, 128), np.float32)
    out[0, :] = 1.0
    return out


def _prep_core_inputs(inputs, b, g):
    import ml_dtypes

    f32 = np.float32
    sl = slice(g * DG, (g + 1) * DG)
    scale = float(Dh) ** -0.5
    maskf = inputs["mask"][b, :, 0].astype(f32)          # (N,) in {0,1}
    km = (maskf - 1.0) * (-NEG)                          # 0 valid, NEG masked
    dn = np.zeros((128, 128), f32)
    np.fill_diagonal(dn, NEG)
    return {
        "xqT": np.ascontiguousarray(inputs["query"][b].T.astype(f32)),
        "xkT": np.ascontiguousarray(inputs["key"][b].T.astype(f32)),
        "xvT": np.ascontiguousarray(inputs["value"][b].T.astype(f32)),
        "wq1": np.ascontiguousarray(inputs["Wq1"].astype(f32)),
        "wk1": np.ascontiguousarray(inputs["Wk1"].astype(f32)),
        "wv1": np.ascontiguousarray(inputs["Wv1"].astype(f32)),
        "bq1": np.ascontiguousarray(
            inputs["bq1"].astype(f32).reshape(HID // 128, 128).T),
        "bk1": np.ascontiguousarray(
            inputs["bk1"].astype(f32).reshape(HID // 128, 128).T),
        "bv1": np.ascontiguousarray(
            inputs["bv1"].astype(f32).reshape(HID // 128, 128).T),
        "wq2": np.ascontiguousarray(inputs["Wq2"][:, sl].astype(f32) * scale),
        "wk2": np.ascontiguousarray(inputs["Wk2"][:, sl].astype(f32)),
        "wv2": np.ascontiguousarray(inputs["Wv2"][:, sl].astype(f32)),
        "bq2": np.ascontiguousarray(
            (inputs["bq2"][sl].astype(f32) * scale).reshape(DG // 128, 128).T),
        "bk2": np.ascontiguousarray(
            inputs["bk2"][sl].astype(f32).reshape(DG // 128, 128).T),
        "bv2r": _row0_pad(inputs["bv2"][sl].astype(f32), 128),
        "onesc": np.ones((128, 1), f32),
        "e0Td": _e0t(),
        "wpb": np.ascontiguousarray(
            inputs["Wp"][sl, :].astype(ml_dtypes.bfloat16)),
        "kmadd": np.ascontiguousarray(km.reshape(N // 128, 128).T),
        "dneg": dn,
    }


def kernel(**inputs):
    import sys
    if "/opt/trn_rl_repo" not in sys.path:
        sys.path.insert(0, "/opt/trn_rl_repo")
    from concourse.bass_utils import run_bass_kernel_spmd

    inputs = {k: np.asarray(v) for k, v in inputs.items()}

    if "nc" not in _CACHE:
        _CACHE["nc"] = _build_nc()
    nc = _CACHE["nc"]

    in_maps = [
        _prep_core_inputs(inputs, c // HG, c % HG) for c in range(NCORES)
    ]

    res = run_bass_kernel_spmd(nc, in_maps, core_ids=list(range(NCORES)))
    results = res.results

    bp = inputs["bp"].astype(np.float32)
    out = np.empty((B, N, OUT_DIM), np.float32)
    for b in range(B):
        acc = results[b * HG]["outT"].astype(np.float32)
        for g in range(1, HG):
            acc = acc + results[b * HG + g]["outT"].astype(np.float32)
        maskf = inputs["mask"][b, :, 0].astype(np.float32)
        out[b] = (acc.T * maskf[:, None]) + bp[None, :]
    return out


# revision 13
# speedup vs baseline: 1.0966x; 1.0966x over previous
"""Bass/Trainium2 kernel for nn_Attention (B=4, N=2048, IN=256, HID=1024,
D=1024, OUT=256, H=8 heads), SPMD over 8 NeuronCores.

Sharding: core c handles batch b = c//2 and head-group g = c%2 (4 heads,
512 of the 1024 inner features).  Layer-1 of each QKV MLP is recomputed on
both cores of a batch (cheap); the output projection is computed per
head-group and the two partial products are summed on the host (plus bias;
the query mask commutes with the projection so it is applied on host too).

Per-core dataflow (laid out so no on-chip transposes are ever needed):
  xT (256,2048) -> L1 feature-major h=(1024,2048) tanh -> L2:
     qT,kT feature-major (512,2048) = 4 head tiles [128,2048]
     v token-major (2048,512) (bias added via rank-1 matmul)
  attention per (head, q-chunk of 1024): S^T tiles [128 k-tok, 1024 q]
     = kT_tile.T @ qT ; key mask enters as the per-partition bias of the Exp
     activation; the no-self-attention diagonal is one [128,128] additive
     DVE op; denominators s = ones^T @ (sum of exp tiles); 1/s is broadcast
     across partitions with an SBUF->SBUF DMA; y^T accumulates in PSUM.
  proj: out^T = Wp_g^T @ (y^T * 1/s) in bf16.
"""

import numpy as np

B, N, IN_DIM, HID, D, OUT_DIM, H = 4, 2048, 256, 1024, 1024, 256, 8
NCORES = 8
HG = 2                 # head groups (cores per batch)
DG = D // HG           # 512 features per group
HEADS_G = H // HG      # 4 heads per core
Dh = D // H            # 128
NEG = -30000.0         # additive mask value (exp underflows to 0)

_CACHE = {}


def _build_nc():
    import concourse.mybir as mybir
    import concourse.tile as tile
    from concourse import bacc
    from contextlib import ExitStack

    dt = mybir.dt
    f32 = dt.float32
    f32r = dt.float32r
    bf16 = dt.bfloat16
    AF = mybir.ActivationFunctionType
    ALU = mybir.AluOpType

    nc = bacc.Bacc("TRN2", target_bir_lowering=False, debug=False)

    # ---- DRAM I/O ----
    xqT = nc.dram_tensor("xqT", [IN_DIM, N], f32r, kind="ExternalInput")
    xkT = nc.dram_tensor("xkT", [IN_DIM, N], f32r, kind="ExternalInput")
    xvT = nc.dram_tensor("xvT", [IN_DIM, N], f32r, kind="ExternalInput")
    wq1 = nc.dram_tensor("wq1", [IN_DIM, HID], f32r, kind="ExternalInput")
    wk1 = nc.dram_tensor("wk1", [IN_DIM, HID], f32r, kind="ExternalInput")
    wv1 = nc.dram_tensor("wv1", [IN_DIM, HID], f32r, kind="ExternalInput")
    bq1 = nc.dram_tensor("bq1", [128, HID // 128], f32, kind="ExternalInput")
    bk1 = nc.dram_tensor("bk1", [128, HID // 128], f32, kind="ExternalInput")
    bv1 = nc.dram_tensor("bv1", [128, HID // 128], f32, kind="ExternalInput")
    wq2 = nc.dram_tensor("wq2", [HID, DG], f32r, kind="ExternalInput")
    wk2 = nc.dram_tensor("wk2", [HID, DG], f32r, kind="ExternalInput")
    wv2 = nc.dram_tensor("wv2", [HID, DG], f32r, kind="ExternalInput")
    bq2 = nc.dram_tensor("bq2", [128, DG // 128], f32, kind="ExternalInput")
    bk2 = nc.dram_tensor("bk2", [128, DG // 128], f32, kind="ExternalInput")
    bv2r = nc.dram_tensor("bv2r", [128, DG], f32r, kind="ExternalInput")
    onesc = nc.dram_tensor("onesc", [128, 128], f32r, kind="ExternalInput")
    e0Td = nc.dram_tensor("e0Td", [128, 128], f32r, kind="ExternalInput")
    wpb = nc.dram_tensor("wpb", [DG, OUT_DIM], bf16, kind="ExternalInput")
    kmadd = nc.dram_tensor("kmadd", [128, N // 128], f32, kind="ExternalInput")
    dneg = nc.dram_tensor("dneg", [128, 128], f32, kind="ExternalInput")
    outT = nc.dram_tensor("outT", [OUT_DIM, N], f32, kind="ExternalOutput")

    KT1 = IN_DIM // 128          # 2  k-tiles in layer 1
    KT2 = HID // 128             # 8  k-tiles in layer 2
    MT1 = HID // 128             # 8  m-tiles in layer 1
    NTOK = N // 128              # 16 token tiles
    NQCH = 2                     # q-chunks
    QW = N // NQCH               # 1024

    with tile.TileContext(nc) as tc, ExitStack() as ctx:
        # pools (PSUM: ps 3x2 banks + psy 1x2 banks = 8 banks)
        ps = ctx.enter_context(tc.tile_pool(name="ps", bufs=3, space="PSUM"))
        psy = ctx.enter_context(tc.tile_pool(name="psy", bufs=1, space="PSUM"))
        singles = ctx.enter_context(tc.tile_pool(name="singles", bufs=1))
        xt_pool = ctx.enter_context(tc.tile_pool(name="xt", bufs=3))
        w1_pool = ctx.enter_context(tc.tile_pool(name="w1", bufs=4))
        w2_pool = ctx.enter_context(tc.tile_pool(name="w2", bufs=8))
        h_pool = ctx.enter_context(tc.tile_pool(name="h", bufs=8))
        qk_pool = ctx.enter_context(tc.tile_pool(name="qk", bufs=8))
        v_pool = ctx.enter_context(tc.tile_pool(name="v", bufs=4))
        pt_pool = ctx.enter_context(tc.tile_pool(name="pt", bufs=4))
        sacc_pool = ctx.enter_context(tc.tile_pool(name="sacc", bufs=2))
        ysc_pool = ctx.enter_context(tc.tile_pool(name="ysc", bufs=4))
        rb_pool = ctx.enter_context(tc.tile_pool(name="rb", bufs=2))
        out_pool = ctx.enter_context(tc.tile_pool(name="out", bufs=1))

        # constants
        ones128 = singles.tile([128, 128], f32r, tag="ones128")
        nc.sync.dma_start(out=ones128, in_=onesc[:, :])
        e0T = singles.tile([128, 128], f32r, tag="e0T")
        nc.sync.dma_start(out=e0T, in_=e0Td[:, :])
        km_sb = singles.tile([128, N // 128], f32, tag="km")
        nc.sync.dma_start(out=km_sb, in_=kmadd[:, :])
        dneg_sb = singles.tile([128, 128], f32, tag="dneg")
        nc.sync.dma_start(out=dneg_sb, in_=dneg[:, :])
        bv2_sb = singles.tile([128, DG], f32r, tag="bv2")
        nc.sync.dma_start(out=bv2_sb, in_=bv2r[:, :])
        wp_sb = singles.tile([128, HEADS_G, OUT_DIM], bf16, tag="wp")
        nc.sync.dma_start(
            out=wp_sb, in_=wpb.rearrange("(h p) o -> p h o", p=128)
        )
        b1_sb = {}
        b2_sb = {}
        for t, (b1d, b2d) in {
            "q": (bq1, bq2), "k": (bk1, bk2), "v": (bv1, None)
        }.items():
            b1_sb[t] = singles.tile([128, HID // 128], f32, tag=f"b1{t}", name=f"b1{t}")
            nc.sync.dma_start(out=b1_sb[t], in_=b1d[:, :])
            if b2d is not None:
                b2_sb[t] = singles.tile([128, DG // 128], f32, tag=f"b2{t}", name=f"b2{t}")
                nc.sync.dma_start(out=b2_sb[t], in_=b2d[:, :])

        # persistent activations
        qT = [qk_pool.tile([128, N], f32r, tag="qk", name=f"qT{i}") for i in range(HEADS_G)]
        kT = [qk_pool.tile([128, N], f32r, tag="qk", name=f"kT{i}") for i in range(HEADS_G)]
        v_sb = [v_pool.tile([128, 4 * DG], bf16, tag="v", name=f"v{i}") for i in range(4)]

        # ---------------- phase A: the three MLPs ----------------
        for t, xd, w1d, w2d in (
            ("q", xqT, wq1, wq2), ("k", xkT, wk1, wk2), ("v", xvT, wv1, wv2)
        ):
            w1_sb = []
            for k in range(KT1):
                w1t = w1_pool.tile([128, HID], f32r, tag="w1")
                nc.sync.dma_start(out=w1t, in_=w1d[k * 128:(k + 1) * 128, :])
                w1_sb.append(w1t)
            w2_sb = []
            for k in range(KT2):
                w2t = w2_pool.tile([128, DG], f32r, tag="w2")
                nc.sync.dma_start(out=w2t, in_=w2d[k * 128:(k + 1) * 128, :])
                w2_sb.append(w2t)

            for th in range(2):                      # token halves of 1024
                tok_sl = slice(th * QW, (th + 1) * QW)
                xts = []
                for k in range(KT1):
                    xt = xt_pool.tile([128, QW], f32r, tag="xt")
                    nc.sync.dma_start(
                        out=xt, in_=xd[k * 128:(k + 1) * 128, tok_sl]
                    )
                    xts.append(xt)
                # layer 1 (feature-major)
                h_sb = []
                for m in range(MT1):
                    p1 = ps.tile([128, QW], f32, tag="ps")
                    for k in range(KT1):
                        for qc in range(QW // 512):
                            nc.tensor.matmul(
                                p1[:, qc * 512:(qc + 1) * 512],
                                w1_sb[k][:, m * 128:(m + 1) * 128],
                                xts[k][:, qc * 512:(qc + 1) * 512],
                                start=(k == 0), stop=(k == KT1 - 1),
                            )
                    ht = h_pool.tile([128, QW], f32r, tag="h")
                    nc.scalar.activation(
                        out=ht, in_=p1, func=AF.Tanh,
                        bias=b1_sb[t][:, m:m + 1], scale=1.0,
                    )
                    h_sb.append(ht)
                # layer 2
                if t in ("q", "k"):
                    dst = qT if t == "q" else kT
                    for m in range(DG // 128):       # head tiles
                        p2 = ps.tile([128, QW], f32, tag="ps")
                        for k in range(KT2):
                            for qc in range(QW // 512):
                                nc.tensor.matmul(
                                    p2[:, qc * 512:(qc + 1) * 512],
                                    w2_sb[k][:, m * 128:(m + 1) * 128],
                                    h_sb[k][:, qc * 512:(qc + 1) * 512],
                                    start=(k == 0), stop=(k == KT2 - 1),
                                )
                        nc.vector.tensor_scalar_add(
                            out=dst[m][:, tok_sl], in0=p2,
                            scalar1=b2_sb[t][:, m:m + 1],
                        )
                else:
                    # v: token-major [tok, feat], bias via rank-1 matmul
                    for tp in range(4):              # pairs of token tiles
                        pv = ps.tile([128, QW], f32, tag="ps")
                        for tt in range(2):
                            sl = slice(tt * 512, (tt + 1) * 512)
                            for k in range(KT2):
                                nc.tensor.matmul(
                                    pv[:, sl],
                                    h_sb[k][:, (tp * 2 + tt) * 128:
                                            (tp * 2 + tt + 1) * 128],
                                    w2_sb[k][:, :],
                                    start=(k == 0), stop=False,
                                )
                            nc.tensor.matmul(
                                pv[:, sl], e0T[:, :], bv2_sb[:, :],
                                start=False, stop=True,
                            )
                        tok0 = th * 8 + tp * 2
                        nc.vector.tensor_copy(
                            out=v_sb[tok0 // 4][
                                :, (tok0 % 4) * 512:(tok0 % 4 + 2) * 512],
                            in_=pv,
                        )

        # ---------------- phase B: attention + projection ----------------
        for qch in range(NQCH):
            q_sl = [slice(qch * QW + i * 512, qch * QW + (i + 1) * 512)
                    for i in range(QW // 512)]
            ysc_tiles = []
            for hd in range(HEADS_G):
                y2 = psy.tile([128, QW], f32, tag="y2")
                sacc = sacc_pool.tile([128, QW], f32r, tag="sacc")
                for kt in range(NTOK):
                    st = ps.tile([128, QW], f32, tag="ps")
                    for i in range(QW // 512):
                        nc.tensor.matmul(
                            st[:, i * 512:(i + 1) * 512],
                            kT[hd][:, kt * 128:(kt + 1) * 128],
                            qT[hd][:, q_sl[i]],
                            start=True, stop=True,
                        )
                    off = kt * 128 - qch * QW
                    if 0 <= off < QW:
                        nc.vector.tensor_tensor(
                            st[:, off:off + 128], st[:, off:off + 128],
                            dneg_sb, ALU.add,
                        )
                    pt = pt_pool.tile([128, QW], bf16, tag="pt")
                    nc.scalar.activation(
                        out=pt, in_=st, func=AF.Exp,
                        bias=km_sb[:, kt:kt + 1], scale=1.0,
                    )
                    if kt == 0:
                        nc.vector.tensor_copy(out=sacc, in_=pt)
                    else:
                        nc.vector.tensor_tensor(sacc, sacc, pt, ALU.add)
                    vt = v_sb[kt // 4][
                        :, (kt % 4) * 512 + hd * 128:
                        (kt % 4) * 512 + (hd + 1) * 128]
                    for i in range(QW // 512):
                        nc.tensor.matmul(
                            y2[:, i * 512:(i + 1) * 512], vt,
                            pt[:, i * 512:(i + 1) * 512],
                            start=(kt == 0), stop=(kt == NTOK - 1),
                        )
                # denominators: all-ones stationary matmul computes the
                # partition-sum of sacc already broadcast to all partitions
                aux = ps.tile([128, QW], f32, tag="ps")
                for i in range(QW // 512):
                    nc.tensor.matmul(
                        aux[:, i * 512:(i + 1) * 512], ones128[:, :],
                        sacc[:, i * 512:(i + 1) * 512],
                        start=True, stop=True,
                    )
                rb2 = rb_pool.tile([128, QW], f32, tag="rb")
                nc.vector.reciprocal(out=rb2, in_=aux)
                ysc = ysc_pool.tile([128, QW], bf16, tag="ysc")
                nc.vector.tensor_tensor(ysc, y2, rb2, ALU.mult)
                ysc_tiles.append(ysc)
            # projection for this q-chunk
            for od in range(OUT_DIM // 128):
                pp = psy.tile([128, QW], f32, tag="y2")
                for i in range(QW // 512):
                    for hd in range(HEADS_G):
                        nc.tensor.matmul(
                            pp[:, i * 512:(i + 1) * 512],
                            wp_sb[:, hd, od * 128:(od + 1) * 128],
                            ysc_tiles[hd][:, i * 512:(i + 1) * 512],
                            start=(hd == 0), stop=(hd == HEADS_G - 1),
                        )
                ot = out_pool.tile([128, QW], f32, tag="out")
                nc.vector.tensor_copy(out=ot, in_=pp)
                nc.sync.dma_start(
                    out=outT[od * 128:(od + 1) * 128,
                             qch * QW:(qch + 1) * QW],
                    in_=ot,
                )

    nc.compile()
    return nc


def _row0_pad(row, nrows):
    out = np.zeros((nrows, row.shape[0]), np.float32)
    out[0] = row
    return out


def _e0t():
    out = np.zeros((128, 128), np.float32)
    out[0, :] = 1.0
    return out


def _prep_core_inputs(inputs, b, g):
    import ml_dtypes

    f32 = np.float32
    sl = slice(g * DG, (g + 1) * DG)
    scale = float(Dh) ** -0.5
    maskf = inputs["mask"][b, :, 0].astype(f32)          # (N,) in {0,1}
    km = (maskf - 1.0) * (-NEG)                          # 0 valid, NEG masked
    dn = np.zeros((128, 128), f32)
    np.fill_diagonal(dn, NEG)
    return {
        "xqT": np.ascontiguousarray(inputs["query"][b].T.astype(f32)),
        "xkT": np.ascontiguousarray(inputs["key"][b].T.astype(f32)),
        "xvT": np.ascontiguousarray(inputs["value"][b].T.astype(f32)),
        "wq1": np.ascontiguousarray(inputs["Wq1"].astype(f32)),
        "wk1": np.ascontiguousarray(inputs["Wk1"].astype(f32)),
        "wv1": np.ascontiguousarray(inputs["Wv1"].astype(f32)),
        "bq1": np.ascontiguousarray(
            inputs["bq1"].astype(f32).reshape(HID // 128, 128).T),
        "bk1": np.ascontiguousarray(
            inputs["bk1"].astype(f32).reshape(HID // 128, 128).T),
        "bv1": np.ascontiguousarray(
            inputs["bv1"].astype(f32).reshape(HID // 128, 128).T),
        "wq2": np.ascontiguousarray(inputs["Wq2"][:, sl].astype(f32) * scale),
        "wk2": np.ascontiguousarray(inputs["Wk2"][:, sl].astype(f32)),
        "wv2": np.ascontiguousarray(inputs["Wv2"][:, sl].astype(f32)),
        "bq2": np.ascontiguousarray(
            (inputs["bq2"][sl].astype(f32) * scale).reshape(DG // 128, 128).T),
        "bk2": np.ascontiguousarray(
            inputs["bk2"][sl].astype(f32).reshape(DG // 128, 128).T),
        "bv2r": _row0_pad(inputs["bv2"][sl].astype(f32), 128),
        "onesc": np.ones((128, 128), f32),
        "e0Td": _e0t(),
        "wpb": np.ascontiguousarray(
            inputs["Wp"][sl, :].astype(ml_dtypes.bfloat16)),
        "kmadd": np.ascontiguousarray(km.reshape(N // 128, 128).T),
        "dneg": dn,
    }


def kernel(**inputs):
    import sys
    if "/opt/trn_rl_repo" not in sys.path:
        sys.path.insert(0, "/opt/trn_rl_repo")
    from concourse.bass_utils import run_bass_kernel_spmd

    inputs = {k: np.asarray(v) for k, v in inputs.items()}

    if "nc" not in _CACHE:
        _CACHE["nc"] = _build_nc()
    nc = _CACHE["nc"]

    in_maps = [
        _prep_core_inputs(inputs, c // HG, c % HG) for c in range(NCORES)
    ]

    res = run_bass_kernel_spmd(nc, in_maps, core_ids=list(range(NCORES)))
    results = res.results

    bp = inputs["bp"].astype(np.float32)
    out = np.empty((B, N, OUT_DIM), np.float32)
    for b in range(B):
        acc = results[b * HG]["outT"].astype(np.float32)
        for g in range(1, HG):
            acc = acc + results[b * HG + g]["outT"].astype(np.float32)
        maskf = inputs["mask"][b, :, 0].astype(np.float32)
        out[b] = (acc.T * maskf[:, None]) + bp[None, :]
    return out


# revision 14
# speedup vs baseline: 1.2511x; 1.1409x over previous
"""Bass/Trainium2 kernel for nn_Attention (B=4, N=2048, IN=256, HID=1024,
D=1024, OUT=256, H=8 heads), SPMD over 8 NeuronCores.

Sharding: core c handles batch b = c//2 and head-group g = c%2 (4 heads,
512 of the 1024 inner features).  Layer-1 of each QKV MLP is recomputed on
both cores of a batch (cheap); the output projection is computed per
head-group and the two partial products are summed on the host (plus bias;
the query mask commutes with the projection so it is applied on host too).

Per-core dataflow (laid out so no on-chip transposes are ever needed):
  xT (256,2048) -> L1 feature-major h=(1024,2048) tanh -> L2:
     qT,kT feature-major (512,2048) = 4 head tiles [128,2048]
     v token-major (2048,512) (bias added via rank-1 matmul)
  attention per (head, q-chunk of 1024): S^T tiles [128 k-tok, 1024 q]
     = kT_tile.T @ qT ; key mask enters as the per-partition bias of the Exp
     activation; the no-self-attention diagonal is one [128,128] additive
     DVE op; denominators s = ones^T @ (sum of exp tiles); 1/s is broadcast
     across partitions with an SBUF->SBUF DMA; y^T accumulates in PSUM.
  proj: out^T = Wp_g^T @ (y^T * 1/s) in bf16.
"""

import numpy as np

B, N, IN_DIM, HID, D, OUT_DIM, H = 4, 2048, 256, 1024, 1024, 256, 8
NCORES = 8
HG = 2                 # head groups (cores per batch)
DG = D // HG           # 512 features per group
HEADS_G = H // HG      # 4 heads per core
Dh = D // H            # 128
NEG = -30000.0         # additive mask value (exp underflows to 0)

_CACHE = {}


def _build_nc():
    import concourse.mybir as mybir
    import concourse.tile as tile
    from concourse import bacc
    from contextlib import ExitStack

    dt = mybir.dt
    f32 = dt.float32
    f32r = dt.float32r
    bf16 = dt.bfloat16
    AF = mybir.ActivationFunctionType
    ALU = mybir.AluOpType

    nc = bacc.Bacc("TRN2", target_bir_lowering=False, debug=False)

    # ---- DRAM I/O ----
    xqT = nc.dram_tensor("xqT", [IN_DIM, N], f32r, kind="ExternalInput")
    xkT = nc.dram_tensor("xkT", [IN_DIM, N], f32r, kind="ExternalInput")
    xvT = nc.dram_tensor("xvT", [IN_DIM, N], f32r, kind="ExternalInput")
    wq1 = nc.dram_tensor("wq1", [IN_DIM, HID], f32r, kind="ExternalInput")
    wk1 = nc.dram_tensor("wk1", [IN_DIM, HID], f32r, kind="ExternalInput")
    wv1 = nc.dram_tensor("wv1", [IN_DIM, HID], f32r, kind="ExternalInput")
    bq1 = nc.dram_tensor("bq1", [128, HID // 128], f32, kind="ExternalInput")
    bk1 = nc.dram_tensor("bk1", [128, HID // 128], f32, kind="ExternalInput")
    bv1 = nc.dram_tensor("bv1", [128, HID // 128], f32, kind="ExternalInput")
    wq2 = nc.dram_tensor("wq2", [HID, DG], f32r, kind="ExternalInput")
    wk2 = nc.dram_tensor("wk2", [HID, DG], f32r, kind="ExternalInput")
    wv2 = nc.dram_tensor("wv2", [HID, DG], f32r, kind="ExternalInput")
    bq2 = nc.dram_tensor("bq2", [128, DG // 128], f32, kind="ExternalInput")
    bk2 = nc.dram_tensor("bk2", [128, DG // 128], f32, kind="ExternalInput")
    bv2r = nc.dram_tensor("bv2r", [128, DG], f32r, kind="ExternalInput")
    onesc = nc.dram_tensor("onesc", [128, 128], bf16, kind="ExternalInput")
    e0Td = nc.dram_tensor("e0Td", [128, 128], f32r, kind="ExternalInput")
    wpb = nc.dram_tensor("wpb", [DG, OUT_DIM], bf16, kind="ExternalInput")
    kmadd = nc.dram_tensor("kmadd", [128, N // 128], f32, kind="ExternalInput")
    dneg = nc.dram_tensor("dneg", [128, 128], f32, kind="ExternalInput")
    outT = nc.dram_tensor("outT", [OUT_DIM, N], f32, kind="ExternalOutput")

    KT1 = IN_DIM // 128          # 2  k-tiles in layer 1
    KT2 = HID // 128             # 8  k-tiles in layer 2
    MT1 = HID // 128             # 8  m-tiles in layer 1
    NTOK = N // 128              # 16 token tiles
    NQCH = 2                     # q-chunks
    QW = N // NQCH               # 1024

    with tile.TileContext(nc) as tc, ExitStack() as ctx:
        # pools (PSUM: ps 3x2 banks + psy 1x2 banks = 8 banks)
        ps = ctx.enter_context(tc.tile_pool(name="ps", bufs=3, space="PSUM"))
        psy = ctx.enter_context(tc.tile_pool(name="psy", bufs=1, space="PSUM"))
        singles = ctx.enter_context(tc.tile_pool(name="singles", bufs=1))
        xt_pool = ctx.enter_context(tc.tile_pool(name="xt", bufs=2))
        w1_pool = ctx.enter_context(tc.tile_pool(name="w1", bufs=3))
        w2_pool = ctx.enter_context(tc.tile_pool(name="w2", bufs=8))
        h_pool = ctx.enter_context(tc.tile_pool(name="h", bufs=8))
        qk_pool = ctx.enter_context(tc.tile_pool(name="qk", bufs=8))
        v_pool = ctx.enter_context(tc.tile_pool(name="v", bufs=4))
        pt_pool = ctx.enter_context(tc.tile_pool(name="pt", bufs=4))
        sacc_pool = ctx.enter_context(tc.tile_pool(name="sacc", bufs=2))
        ysc_pool = ctx.enter_context(tc.tile_pool(name="ysc", bufs=4))
        rb_pool = ctx.enter_context(tc.tile_pool(name="rb", bufs=2))
        y2s_pool = ctx.enter_context(tc.tile_pool(name="y2s", bufs=2))
        out_pool = ctx.enter_context(tc.tile_pool(name="out", bufs=1))

        # constants
        ones128 = singles.tile([128, 128], bf16, tag="ones128")
        nc.sync.dma_start(out=ones128, in_=onesc[:, :])
        e0T = singles.tile([128, 128], f32r, tag="e0T")
        nc.sync.dma_start(out=e0T, in_=e0Td[:, :])
        km_sb = singles.tile([128, N // 128], f32, tag="km")
        nc.sync.dma_start(out=km_sb, in_=kmadd[:, :])
        dneg_sb = singles.tile([128, 128], f32, tag="dneg")
        nc.sync.dma_start(out=dneg_sb, in_=dneg[:, :])
        bv2_sb = singles.tile([128, DG], f32r, tag="bv2")
        nc.sync.dma_start(out=bv2_sb, in_=bv2r[:, :])
        wp_sb = singles.tile([128, HEADS_G, OUT_DIM], bf16, tag="wp")
        nc.sync.dma_start(
            out=wp_sb, in_=wpb.rearrange("(h p) o -> p h o", p=128)
        )
        b1_sb = {}
        b2_sb = {}
        for t, (b1d, b2d) in {
            "q": (bq1, bq2), "k": (bk1, bk2), "v": (bv1, None)
        }.items():
            b1_sb[t] = singles.tile([128, HID // 128], f32, tag=f"b1{t}", name=f"b1{t}")
            nc.sync.dma_start(out=b1_sb[t], in_=b1d[:, :])
            if b2d is not None:
                b2_sb[t] = singles.tile([128, DG // 128], f32, tag=f"b2{t}", name=f"b2{t}")
                nc.sync.dma_start(out=b2_sb[t], in_=b2d[:, :])

        # persistent activations
        qT = [qk_pool.tile([128, N], f32r, tag="qk", name=f"qT{i}") for i in range(HEADS_G)]
        kT = [qk_pool.tile([128, N], f32r, tag="qk", name=f"kT{i}") for i in range(HEADS_G)]
        v_sb = [v_pool.tile([128, 4 * DG], bf16, tag="v", name=f"v{i}") for i in range(4)]

        # ---------------- phase A: the three MLPs ----------------
        for t, xd, w1d, w2d in (
            ("q", xqT, wq1, wq2), ("k", xkT, wk1, wk2), ("v", xvT, wv1, wv2)
        ):
            w1_sb = []
            for k in range(KT1):
                w1t = w1_pool.tile([128, HID], f32r, tag="w1")
                nc.sync.dma_start(out=w1t, in_=w1d[k * 128:(k + 1) * 128, :])
                w1_sb.append(w1t)
            w2_sb = []
            for k in range(KT2):
                w2t = w2_pool.tile([128, DG], f32r, tag="w2")
                nc.sync.dma_start(out=w2t, in_=w2d[k * 128:(k + 1) * 128, :])
                w2_sb.append(w2t)

            for th in range(2):                      # token halves of 1024
                tok_sl = slice(th * QW, (th + 1) * QW)
                xts = []
                for k in range(KT1):
                    xt = xt_pool.tile([128, QW], f32r, tag="xt")
                    nc.sync.dma_start(
                        out=xt, in_=xd[k * 128:(k + 1) * 128, tok_sl]
                    )
                    xts.append(xt)
                # layer 1 (feature-major)
                h_sb = []
                for m in range(MT1):
                    p1 = ps.tile([128, QW], f32, tag="ps")
                    for k in range(KT1):
                        for qc in range(QW // 512):
                            nc.tensor.matmul(
                                p1[:, qc * 512:(qc + 1) * 512],
                                w1_sb[k][:, m * 128:(m + 1) * 128],
                                xts[k][:, qc * 512:(qc + 1) * 512],
                                start=(k == 0), stop=(k == KT1 - 1),
                            )
                    ht = h_pool.tile([128, QW], f32r, tag="h")
                    nc.scalar.activation(
                        out=ht, in_=p1, func=AF.Tanh,
                        bias=b1_sb[t][:, m:m + 1], scale=1.0,
                    )
                    h_sb.append(ht)
                # layer 2
                if t in ("q", "k"):
                    dst = qT if t == "q" else kT
                    for m in range(DG // 128):       # head tiles
                        p2 = ps.tile([128, QW], f32, tag="ps")
                        for k in range(KT2):
                            for qc in range(QW // 512):
                                nc.tensor.matmul(
                                    p2[:, qc * 512:(qc + 1) * 512],
                                    w2_sb[k][:, m * 128:(m + 1) * 128],
                                    h_sb[k][:, qc * 512:(qc + 1) * 512],
                                    start=(k == 0), stop=(k == KT2 - 1),
                                )
                        nc.vector.tensor_scalar_add(
                            out=dst[m][:, tok_sl], in0=p2,
                            scalar1=b2_sb[t][:, m:m + 1],
                        )
                else:
                    # v: token-major [tok, feat], bias via rank-1 matmul
                    for tp in range(4):              # pairs of token tiles
                        pv = ps.tile([128, QW], f32, tag="ps")
                        for tt in range(2):
                            sl = slice(tt * 512, (tt + 1) * 512)
                            for k in range(KT2):
                                nc.tensor.matmul(
                                    pv[:, sl],
                                    h_sb[k][:, (tp * 2 + tt) * 128:
                                            (tp * 2 + tt + 1) * 128],
                                    w2_sb[k][:, :],
                                    start=(k == 0), stop=False,
                                )
                            nc.tensor.matmul(
                                pv[:, sl], e0T[:, :], bv2_sb[:, :],
                                start=False, stop=True,
                            )
                        tok0 = th * 8 + tp * 2
                        nc.vector.tensor_copy(
                            out=v_sb[tok0 // 4][
                                :, (tok0 % 4) * 512:(tok0 % 4 + 2) * 512],
                            in_=pv,
                        )

        # ---------------- phase B: attention + projection ----------------
        for qch in range(NQCH):
            q_sl = [slice(qch * QW + i * 512, qch * QW + (i + 1) * 512)
                    for i in range(QW // 512)]
            ysc_tiles = []
            for hd in range(HEADS_G):
                y2 = psy.tile([128, QW], f32, tag="y2")
                sacc = sacc_pool.tile([128, QW], bf16, tag="sacc")
                for kt in range(NTOK):
                    st = ps.tile([128, QW], f32, tag="ps")
                    for i in range(QW // 512):
                        nc.tensor.matmul(
                            st[:, i * 512:(i + 1) * 512],
                            kT[hd][:, kt * 128:(kt + 1) * 128],
                            qT[hd][:, q_sl[i]],
                            start=True, stop=True,
                        )
                    off = kt * 128 - qch * QW
                    if 0 <= off < QW:
                        nc.vector.tensor_tensor(
                            st[:, off:off + 128], st[:, off:off + 128],
                            dneg_sb, ALU.add,
                        )
                    pt = pt_pool.tile([128, QW], bf16, tag="pt")
                    nc.scalar.activation(
                        out=pt, in_=st, func=AF.Exp,
                        bias=km_sb[:, kt:kt + 1], scale=1.0,
                    )
                    if kt == 0:
                        nc.vector.tensor_copy(out=sacc, in_=pt)
                    else:
                        nc.vector.tensor_tensor(sacc, sacc, pt, ALU.add)
                    vt = v_sb[kt // 4][
                        :, (kt % 4) * 512 + hd * 128:
                        (kt % 4) * 512 + (hd + 1) * 128]
                    for i in range(QW // 512):
                        nc.tensor.matmul(
                            y2[:, i * 512:(i + 1) * 512], vt,
                            pt[:, i * 512:(i + 1) * 512],
                            start=(kt == 0), stop=(kt == NTOK - 1),
                        )
                # free the y2 PSUM slot immediately: copy to SBUF, then
                # normalize off the PE critical path
                y2s = y2s_pool.tile([128, QW], f32, tag="y2s")
                nc.vector.tensor_copy(out=y2s, in_=y2)
                # denominators: all-ones stationary matmul computes the
                # partition-sum of sacc already broadcast to all partitions
                aux = ps.tile([128, QW], f32, tag="ps")
                for i in range(QW // 512):
                    nc.tensor.matmul(
                        aux[:, i * 512:(i + 1) * 512], ones128[:, :],
                        sacc[:, i * 512:(i + 1) * 512],
                        start=True, stop=True,
                    )
                rb2 = rb_pool.tile([128, QW], f32, tag="rb")
                nc.vector.reciprocal(out=rb2, in_=aux)
                ysc = ysc_pool.tile([128, QW], bf16, tag="ysc")
                nc.vector.tensor_tensor(ysc, y2s, rb2, ALU.mult)
                ysc_tiles.append(ysc)
            # projection for this q-chunk
            for od in range(OUT_DIM // 128):
                pp = psy.tile([128, QW], f32, tag="y2")
                for i in range(QW // 512):
                    for hd in range(HEADS_G):
                        nc.tensor.matmul(
                            pp[:, i * 512:(i + 1) * 512],
                            wp_sb[:, hd, od * 128:(od + 1) * 128],
                            ysc_tiles[hd][:, i * 512:(i + 1) * 512],
                            start=(hd == 0), stop=(hd == HEADS_G - 1),
                        )
                ot = out_pool.tile([128, QW], f32, tag="out")
                nc.vector.tensor_copy(out=ot, in_=pp)
                nc.sync.dma_start(
                    out=outT[od * 128:(od + 1) * 128,
                             qch * QW:(qch + 1) * QW],
                    in_=ot,
                )

    nc.compile()
    return nc


def _row0_pad(row, nrows):
    out = np.zeros((nrows, row.shape[0]), np.float32)
    out[0] = row
    return out


def _e0t():
    out = np.zeros((128, 128), np.float32)
    out[0, :] = 1.0
    return out


def _prep_core_inputs(inputs, b, g):
    import ml_dtypes

    f32 = np.float32
    sl = slice(g * DG, (g + 1) * DG)
    scale = float(Dh) ** -0.5
    maskf = inputs["mask"][b, :, 0].astype(f32)          # (N,) in {0,1}
    km = (maskf - 1.0) * (-NEG)                          # 0 valid, NEG masked
    dn = np.zeros((128, 128), f32)
    np.fill_diagonal(dn, NEG)
    return {
        "xqT": np.ascontiguousarray(inputs["query"][b].T.astype(f32)),
        "xkT": np.ascontiguousarray(inputs["key"][b].T.astype(f32)),
        "xvT": np.ascontiguousarray(inputs["value"][b].T.astype(f32)),
        "wq1": np.ascontiguousarray(inputs["Wq1"].astype(f32)),
        "wk1": np.ascontiguousarray(inputs["Wk1"].astype(f32)),
        "wv1": np.ascontiguousarray(inputs["Wv1"].astype(f32)),
        "bq1": np.ascontiguousarray(
            inputs["bq1"].astype(f32).reshape(HID // 128, 128).T),
        "bk1": np.ascontiguousarray(
            inputs["bk1"].astype(f32).reshape(HID // 128, 128).T),
        "bv1": np.ascontiguousarray(
            inputs["bv1"].astype(f32).reshape(HID // 128, 128).T),
        "wq2": np.ascontiguousarray(inputs["Wq2"][:, sl].astype(f32) * scale),
        "wk2": np.ascontiguousarray(inputs["Wk2"][:, sl].astype(f32)),
        "wv2": np.ascontiguousarray(inputs["Wv2"][:, sl].astype(f32)),
        "bq2": np.ascontiguousarray(
            (inputs["bq2"][sl].astype(f32) * scale).reshape(DG // 128, 128).T),
        "bk2": np.ascontiguousarray(
            inputs["bk2"][sl].astype(f32).reshape(DG // 128, 128).T),
        "bv2r": _row0_pad(inputs["bv2"][sl].astype(f32), 128),
        "onesc": np.ones((128, 128), ml_dtypes.bfloat16),
        "e0Td": _e0t(),
        "wpb": np.ascontiguousarray(
            inputs["Wp"][sl, :].astype(ml_dtypes.bfloat16)),
        "kmadd": np.ascontiguousarray(km.reshape(N // 128, 128).T),
        "dneg": dn,
    }


def kernel(**inputs):
    import sys
    if "/opt/trn_rl_repo" not in sys.path:
        sys.path.insert(0, "/opt/trn_rl_repo")
    from concourse.bass_utils import run_bass_kernel_spmd

    inputs = {k: np.asarray(v) for k, v in inputs.items()}

    if "nc" not in _CACHE:
        _CACHE["nc"] = _build_nc()
    nc = _CACHE["nc"]

    in_maps = [
        _prep_core_inputs(inputs, c // HG, c % HG) for c in range(NCORES)
    ]

    res = run_bass_kernel_spmd(nc, in_maps, core_ids=list(range(NCORES)))
    results = res.results

    bp = inputs["bp"].astype(np.float32)
    out = np.empty((B, N, OUT_DIM), np.float32)
    for b in range(B):
        acc = results[b * HG]["outT"].astype(np.float32)
        for g in range(1, HG):
            acc = acc + results[b * HG + g]["outT"].astype(np.float32)
        maskf = inputs["mask"][b, :, 0].astype(np.float32)
        out[b] = (acc.T * maskf[:, None]) + bp[None, :]
    return out
